# revision 1
# baseline (speedup 1.0000x reference)
"""Bass/Trainium2 kernel for nn_BiChannelAttention (single-query local-window attention).

Math (per batch b, head h, with S=2049, window W=256, cutoff=S-W=1793):
  Positions before the cutoff receive a -1e6 additive mask, so after softmax their
  weight is exactly 0.0 in fp32 (exp underflows). Only the last W positions matter.

  For window rows X [W, 128] (last 255 cache rows + content row):
    q   = cnt_h @ Wq_h                      (128)
    kq  = (Wk_h/sqrt(128))^T q              (128)      <- folds Wk into q
    sc  = X kq  (+ per-position bias)       (W)        <- column-major on chip
    a   = exp(sc)          (no max-subtraction needed: unmasked scores are O(1))
    xa  = X^T a / sum(a)                    (128)
    out = Wv_h^T xa + cnt_h                 (128)

Sharding: tensor-parallel over heads, 2 heads per core x 8 cores. Each core reads
only its heads' weight slices and window slices (~2.2 MB).
"""

import sys
import numpy as np

for _p in ("/opt/trn_rl_repo", "/root/.axon_site/_ro/trn_rl_repo"):
    if _p not in sys.path:
        sys.path.insert(0, _p)

import concourse.bass as bass
import concourse.bacc as bacc
import concourse.mybir as mybir
from concourse.tile import TileContext
from concourse.bass_utils import run_bass_kernel_spmd

F32 = mybir.dt.float32
P = 128          # partitions / head_dim
B = 8            # batch
H = 16           # heads total
HPC = 2          # heads per core
NCORES = 8
T = 2048
S = T + 1
W = 256          # local attention window
CUTOFF = S - W   # 1793
NEG = -1000000.0

_NC_CACHE = {}


def _build_nc():
    nc = bacc.Bacc(None, target_bir_lowering=False, debug=False)
    # packed constants along the free dim: ident | ones | bias | cnt | (wq,wkt,wv) x HPC
    CK = 2 * P + 2 * B + HPC * B + 3 * HPC * P
    x_in = nc.declare_dram_parameter("x", [B, HPC, W, P], F32, isOutput=False)
    consts_in = nc.declare_dram_parameter("consts", [P, CK], F32, isOutput=False)
    out_t = nc.declare_dram_parameter("out", [HPC, P, B], F32, isOutput=True)

    with TileContext(nc) as tc:
        with (
            tc.tile_pool(name="const", bufs=1) as cpool,
            tc.tile_pool(name="xin", bufs=10) as xpool,
            tc.tile_pool(name="xt", bufs=10) as xtpool,
            tc.tile_pool(name="small", bufs=2) as spool,
            tc.tile_pool(name="ps_t", bufs=2, space="PSUM") as pst,
            tc.tile_pool(name="ps_qk", bufs=2, space="PSUM") as psqk,
            tc.tile_pool(name="ps_at", bufs=2, space="PSUM") as psat,
            tc.tile_pool(name="ps_xo", bufs=2, space="PSUM") as psxo,
        ):
            consts = cpool.tile([P, CK], F32, tag="consts")
            nc.sync.dma_start(out=consts[:, :], in_=consts_in[:, :])
            o = 0
            ident = consts[:, o:o + P]; o += P
            ones = consts[:, o:o + P]; o += P
            biasT = consts[:, o:o + 2 * B]; o += 2 * B
            cntT = consts[:, o:o + HPC * B]; o += HPC * B
            wq, wkt, wv = [], [], []
            for j in range(HPC):
                wq.append(consts[:, o:o + P]); o += P
                wkt.append(consts[:, o:o + P]); o += P
                wv.append(consts[:, o:o + P]); o += P

            for j in range(HPC):
                cnt_j = cntT[:, j * B:(j + 1) * B]

                # q for all 8 batches: q[e,b] = sum_d Wq[d,e] cnt[d,b]
                qk_ps = psqk.tile([P, 2 * B], F32, tag="qk")
                nc.tensor.matmul(qk_ps[:, 0:B], wq[j], cnt_j, start=True, stop=True)
                q_sb = spool.tile([P, B], F32, tag="q")
                nc.vector.tensor_copy(q_sb[:, :], qk_ps[:, 0:B])
                # kq[d,b] = sum_e WkT[e,d] q[e,b]   (WkT pre-scaled by 1/sqrt(128))
                nc.tensor.matmul(qk_ps[:, B:2 * B], wkt[j], q_sb[:, :], start=True, stop=True)
                kq_sb = spool.tile([P, B], F32, tag="kq")
                nc.vector.tensor_copy(kq_sb[:, :], qk_ps[:, B:2 * B])

                at_ps = psat.tile([P, 3 * B], F32, tag="at")  # scores [0:16], denom [16:24]
                xo_ps = psxo.tile([P, 2 * B], F32, tag="xo")  # xa [0:8], out [8:16]

                x0s, x1s, xt0s, xt1s = [], [], [], []
                for b in range(B):
                    x0 = xpool.tile([P, P], F32, tag="x0")
                    nc.sync.dma_start(out=x0[:, :], in_=x_in[b, j, 0:P, :])
                    x1 = xpool.tile([P, P], F32, tag="x1")
                    nc.sync.dma_start(out=x1[:, :], in_=x_in[b, j, P:W, :])
                    xt_ps = pst.tile([P, 2 * P], F32, tag="xtp")
                    nc.tensor.transpose(xt_ps[:, 0:P], x0[:, :], ident)
                    nc.tensor.transpose(xt_ps[:, P:2 * P], x1[:, :], ident)
                    xt0 = xtpool.tile([P, P], F32, tag="xt0")
                    nc.vector.tensor_copy(xt0[:, :], xt_ps[:, 0:P])
                    xt1 = xtpool.tile([P, P], F32, tag="xt1")
                    nc.scalar.copy(xt1[:, :], xt_ps[:, P:2 * P])
                    # scores: column [s,1] per (tile, b) -> at_ps col jt*8+b
                    nc.tensor.matmul(at_ps[:, b:b + 1], xt0[:, :], kq_sb[:, b:b + 1], start=True, stop=True)
                    nc.tensor.matmul(at_ps[:, B + b:B + b + 1], xt1[:, :], kq_sb[:, b:b + 1], start=True, stop=True)
                    x0s.append(x0); x1s.append(x1); xt0s.append(xt0); xt1s.append(xt1)

                # bias add + exp for all 16 score columns at once
                att_pre = spool.tile([P, 2 * B], F32, tag="att_pre")
                nc.vector.tensor_add(att_pre[:, :], at_ps[:, 0:2 * B], biasT)
                att = spool.tile([P, 2 * B], F32, tag="att")
                nc.scalar.activation(att[:, :], att_pre[:, :], mybir.ActivationFunctionType.Exp)

                # denominator broadcast over partitions: accumulate both s-tiles on PE
                nc.tensor.matmul(at_ps[:, 2 * B:3 * B], ones, att[:, 0:B], start=True, stop=False)
                nc.tensor.matmul(at_ps[:, 2 * B:3 * B], ones, att[:, B:2 * B], start=False, stop=True)
                rec = spool.tile([P, B], F32, tag="rec")
                nc.vector.reciprocal(rec[:, :], at_ps[:, 2 * B:3 * B])

                # xa[d,b] = sum_s X[s,d] a[s,b]  (accumulate the two s-tiles)
                for b in range(B):
                    nc.tensor.matmul(xo_ps[:, b:b + 1], x0s[b][:, :], att[:, b:b + 1], start=True, stop=False)
                    nc.tensor.matmul(xo_ps[:, b:b + 1], x1s[b][:, :], att[:, B + b:B + b + 1], start=False, stop=True)
                xa_sb = spool.tile([P, B], F32, tag="xa")
                nc.vector.tensor_mul(xa_sb[:, :], xo_ps[:, 0:B], rec[:, :])

                # out[e,b] = sum_d Wv[d,e] xa[d,b]; residual add; store
                nc.tensor.matmul(xo_ps[:, B:2 * B], wv[j], xa_sb[:, :], start=True, stop=True)
                fin = spool.tile([P, B], F32, tag="fin")
                nc.vector.tensor_add(fin[:, :], xo_ps[:, B:2 * B], cnt_j)
                nc.sync.dma_start(out=out_t[j, :, :], in_=fin[:, :])
    nc.finalize()
    return nc


def _get_nc():
    if "nc" not in _NC_CACHE:
        _NC_CACHE["nc"] = _build_nc()
    return _NC_CACHE["nc"]


def _pos_bias_f32():
    """t5_position_bucket exactly as the reference computes it (same jnp ops on the
    in-process default jax backend), sliced to the window."""
    if "pos" not in _NC_CACHE:
        import jax.numpy as jnp
        NUM_BUCKETS, MAX_DISTANCE = 32, 128
        n = (S - 1) - jnp.arange(S)
        max_exact = NUM_BUCKETS // 2
        is_small = n < max_exact
        large = max_exact + (
            jnp.log(jnp.maximum(n, 1).astype(jnp.float32) / max_exact)
            / np.log(MAX_DISTANCE / max_exact)
            * (NUM_BUCKETS - max_exact)
        ).astype(jnp.int32)
        large = jnp.minimum(large, NUM_BUCKETS - 1)
        pos = jnp.where(is_small, n, large).astype(jnp.float32)
        _NC_CACHE["pos"] = np.asarray(pos)[CUTOFF:]  # [W]
    return _NC_CACHE["pos"]


def kernel(**inputs) -> np.ndarray:
    t = int(np.asarray(inputs["t"]))
    assert t == T, f"kernel hardcoded for t={T}, got {t}"
    content_t = np.ascontiguousarray(np.asarray(inputs["content_t"], dtype=np.float32))
    time_mask = np.asarray(inputs["time_mask"])
    cache = np.asarray(inputs["cache"], dtype=np.float32)
    Wq = np.asarray(inputs["Wq"], dtype=np.float32)
    Wk = np.asarray(inputs["Wk"], dtype=np.float32)
    Wv = np.asarray(inputs["Wv"], dtype=np.float32)
    pos_param = np.float32(np.asarray(inputs["pos_param"]))

    # Per-position additive bias for the window: -pos_param*bucket only.
    # The reference's masked_fill sequence (1->0, then every 0->NEG) sets ALL
    # positions to NEG, a uniform shift softmax cancels -- time_mask is a no-op.
    del time_mask
    pos = _pos_bias_f32()                                   # [W]
    posb = (-pos_param * pos).astype(np.float32)            # [W]
    bias_col = posb.reshape(2, P).transpose(1, 0)           # [p, jt]
    bias_t = np.ascontiguousarray(
        np.broadcast_to(bias_col[:, :, None], (P, 2, B)).reshape(P, 2 * B)
    )  # [p, jt*8+b]

    win = cache[:, CUTOFF:T, :].reshape(B, W - 1, H, P)      # [B, 255, H, 128]
    cnt_h = content_t.reshape(B, H, P)                       # [B, H, 128]
    wkt_full = (Wk.transpose(0, 2, 1) / np.float32(np.sqrt(128.0))).astype(np.float32)

    ones = np.ones((P, P), np.float32)
    ident = np.eye(P, dtype=np.float32)

    in_maps = []
    for c in range(NCORES):
        h0 = HPC * c
        x_host = np.empty((B, HPC, W, P), np.float32)
        for j in range(HPC):
            x_host[:, j, : W - 1, :] = win[:, :, h0 + j, :]
            x_host[:, j, W - 1, :] = cnt_h[:, h0 + j, :]
        cnt_host = np.ascontiguousarray(
            cnt_h[:, h0:h0 + HPC, :].transpose(2, 1, 0).reshape(P, HPC * B)
        )  # [d, j*8+b]
        blocks = [ident, ones, bias_t, cnt_host]
        for j in range(HPC):
            blocks += [Wq[h0 + j], wkt_full[h0 + j], Wv[h0 + j]]
        consts_host = np.ascontiguousarray(np.concatenate(blocks, axis=1), dtype=np.float32)
        in_maps.append({"x": x_host, "consts": consts_host})

    nc = _get_nc()
    res = run_bass_kernel_spmd(nc, in_maps, list(range(NCORES)), **_RUN_KWARGS)
    _NC_CACHE["last_results"] = res
    outs = np.stack([np.asarray(res.results[c]["out"]) for c in range(NCORES)])
    # outs: [core, j, d, b] -> out_full[b, (2c+j)*128 + d]
    out_full = outs.transpose(3, 0, 1, 2).reshape(B, H * P)
    return out_full.astype(np.float32)


_RUN_KWARGS = {}  # test harness may set {"trace": True, "tmpdir": ...}



# revision 6
# speedup vs baseline: 3.1059x; 3.1059x over previous
"""Bass/Trainium2 kernel for nn_BiChannelAttention (single-query local-window attention).

Math (per batch b, head h, S=2049, window W=256, cutoff=S-W=1793):
  Every in-window position carries the same -1e6 time-mask shift (the reference's
  masked_fill collapses to a uniform constant), which softmax cancels; positions
  before the cutoff are -1e6 relative => weight exactly 0 in fp32. Only the last
  W positions (255 cache rows + the content row) matter.

  Per (b,h) with window rows X [W,128]:
    q   = Wq^T cnt + bq                       (on device)
    kq  = (Wk/sqrt(128)) q                    (on device; the q.bk term is an
                                               s-constant and cancels in softmax)
    sc  = X kq - pos_param*bucket(s)          (PE matvec per s-half, s on partitions)
    a   = exp(sc)  (ACT, per-partition bias fused; no max-subtraction needed)
    av  = X^T a / sum(a)
    out = Wv^T av + bv + cnt                  (bv folded into the residual const)

Layouts: everything bf16 on the PE (fp32 matmuls are 4 cyc/row and disable FWL).
The host ships the window in BOTH layouts (natural [s,d] for the AV stationary,
transposed [d,s] for the scores stationary) -- 2 MB/core -- which is cheaper than
32 on-chip PE transposes + PSUM copies. Stationaries are per-(pair,half) 128-col
tiles (FWL ~53ns each); moving operands are 1-8 columns. A short dummy-matmul
warmup keeps the PE HAM clock at 2.4 GHz by the time real work lands, and a dummy
exp preloads the ACT table. DMA instruction count is minimized (~7 in, 1 out,
~700ns issue each) and split across the two HWDGE queues (sync + scalar).

Sharding: tensor-parallel over heads, 2 heads per core x 8 cores.
"""

import sys
import numpy as np

for _p in ("/opt/trn_rl_repo", "/root/.axon_site/_ro/trn_rl_repo"):
    if _p not in sys.path:
        sys.path.insert(0, _p)

import ml_dtypes
import concourse.bass as bass
import concourse.bacc as bacc
import concourse.mybir as mybir
from concourse.tile import TileContext
from concourse.bass_utils import run_bass_kernel_spmd

F32 = mybir.dt.float32
BF16 = mybir.dt.bfloat16
NPBF = ml_dtypes.bfloat16
P = 128          # partitions / head_dim
B = 8            # batch
H = 16           # heads total
HPC = 2          # heads per core
NCORES = 8
T = 2048
S = T + 1
W = 256          # local attention window
CUTOFF = S - W   # 1793
NWARM = 18       # PE warmup matmuls (HAM ramp ~3.4us)

_NC_CACHE = {}

# cbw (bf16) column map: [cnt 16][w: per head wq|wkt|wv 768][ones 128]
CB0 = 0                      # cnt bf16, cols j*8+b
CW0 = HPC * B                # weights
CO0 = CW0 + HPC * 3 * P      # ones block (col of ones + row of ones)
CBW = CO0 + P
# cf (f32) column map: [cnt+bv 16][bias0][bias1][bq0][bq1]
CF_BIAS = HPC * B
CF_BQ = CF_BIAS + 2
CFW = CF_BQ + HPC


def _build_nc():
    nc = bacc.Bacc(None, target_bir_lowering=False, debug=False)
    xt_in = nc.declare_dram_parameter("xt", [HPC, P, B * W], BF16, isOutput=False)
    xn_in = nc.declare_dram_parameter("xn", [HPC, P, B * W], BF16, isOutput=False)
    cbw_in = nc.declare_dram_parameter("cbw", [P, CBW], BF16, isOutput=False)
    cf_in = nc.declare_dram_parameter("cf", [P, CFW], F32, isOutput=False)
    out_t = nc.declare_dram_parameter("out", [P, HPC * B], F32, isOutput=True)

    with TileContext(nc) as tc:
        with (
            nc.allow_low_precision(reason="bf16 pipeline validated vs reference"),
            tc.tile_pool(name="big", bufs=1) as bigp,
            tc.tile_pool(name="small", bufs=14) as spool,
            tc.tile_pool(name="ps_qk", bufs=2, space="PSUM") as psQK,
            tc.tile_pool(name="ps_z", bufs=1, space="PSUM") as psZ,
            tc.tile_pool(name="ps_s", bufs=2, space="PSUM") as psS,
            tc.tile_pool(name="ps_v", bufs=2, space="PSUM") as psV,
            tc.tile_pool(name="ps_o", bufs=1, space="PSUM") as psO,
        ):
            # ---- DMAs (sync queue: cbw, xt0, xt1, out; act queue: cf, xn0, xn1)
            cbw = bigp.tile([P, CBW], BF16, tag="cbw")
            nc.sync.dma_start(out=cbw[:, :], in_=cbw_in[:, :])
            cf = bigp.tile([P, CFW], F32, tag="cf")
            nc.scalar.dma_start(out=cf[:, :], in_=cf_in[:, :])
            xt = []
            xn = []
            for j in range(HPC):
                t_ = bigp.tile([P, B * W], BF16, tag=f"xt{j}")
                nc.sync.dma_start(out=t_[:, :], in_=xt_in[j, :, :])
                xt.append(t_)
                n_ = bigp.tile([P, B * W], BF16, tag=f"xn{j}")
                nc.scalar.dma_start(out=n_[:, :], in_=xn_in[j, :, :])
                xn.append(n_)

            cnt_bf = cbw[:, CB0:CB0 + HPC * B]
            wq = [cbw[:, CW0 + j * 3 * P:CW0 + j * 3 * P + P] for j in range(HPC)]
            wkt = [cbw[:, CW0 + j * 3 * P + P:CW0 + j * 3 * P + 2 * P] for j in range(HPC)]
            wv = [cbw[:, CW0 + j * 3 * P + 2 * P:CW0 + j * 3 * P + 3 * P] for j in range(HPC)]
            ones_col = cbw[:, CO0:CO0 + 1]          # [128,1] of 1.0 (bf16)
            ones_row = cbw[0:1, CO0:CO0 + P]        # [1,128] of 1.0 (bf16)

            # ---- PE warmup while DMAs land: dummy matmuls on a zeroed tile.
            wdat = spool.tile([P, P], BF16, tag="warm")
            nc.vector.memset(wdat[:, :], 0.0)
            wps = psO.tile([P, P], F32, tag="ow")
            for i in range(NWARM):
                nc.tensor.matmul(wps[:, :], wdat[:, :], wdat[:, :],
                                 start=True, stop=True)
            # preload the ACT exp table with a dummy activation
            wact = spool.tile([P, 1], F32, tag="warm_act")
            nc.scalar.activation(wact[:, :], wdat[:, 0:1],
                                 mybir.ActivationFunctionType.Exp)

            z_ps = psZ.tile([1, HPC * B], F32, tag="zrb")     # softmax denominators
            att = []
            avn = []
            for j in range(HPC):
                # q = Wq^T cnt (+bq later), kq = (Wk^T scaled)^T q = Wk q / sqrt(hd)
                q_ps = psQK.tile([P, B], F32, tag="qk")
                nc.tensor.matmul(q_ps[:, :], wq[j], cnt_bf[:, j * B:(j + 1) * B],
                                 start=True, stop=True)
                q_sb = spool.tile([P, B], BF16, tag=f"qsb{j}")
                nc.vector.tensor_scalar_add(q_sb[:, :], q_ps[:, :],
                                            cf[:, CF_BQ + j:CF_BQ + j + 1])
                kq_ps = psQK.tile([P, B], F32, tag="qk")
                nc.tensor.matmul(kq_ps[:, :], wkt[j], q_sb[:, :],
                                 start=True, stop=True)
                kq_sb = spool.tile([P, B], BF16, tag=f"kqsb{j}")
                nc.vector.tensor_copy(kq_sb[:, :], kq_ps[:, :])

                # scores: per (b, s-half) stationary Xt tile, 1-col moving kq
                sc = psS.tile([P, 2 * B], F32, tag="sc")
                for b in range(B):
                    nc.tensor.matmul(sc[:, b:b + 1],
                                     xt[j][:, b * W:b * W + P],
                                     kq_sb[:, b:b + 1], start=True, stop=True)
                    nc.tensor.matmul(sc[:, B + b:B + b + 1],
                                     xt[j][:, b * W + P:b * W + 2 * P],
                                     kq_sb[:, b:b + 1], start=True, stop=True)

                # att = exp(sc + bias(s)) -- per-partition bias, one ACT per half
                a_sb = spool.tile([P, 2 * B], BF16, tag=f"att{j}")
                nc.scalar.activation(a_sb[:, 0:B], sc[:, 0:B],
                                     mybir.ActivationFunctionType.Exp,
                                     bias=cf[:, CF_BIAS:CF_BIAS + 1])
                nc.scalar.activation(a_sb[:, B:2 * B], sc[:, B:2 * B],
                                     mybir.ActivationFunctionType.Exp,
                                     bias=cf[:, CF_BIAS + 1:CF_BIAS + 2])
                att.append(a_sb)

                # denominator: ones^T att (accumulate both halves)
                nc.tensor.matmul(z_ps[:, j * B:(j + 1) * B], ones_col,
                                 a_sb[:, 0:B], start=True, stop=False)
                nc.tensor.matmul(z_ps[:, j * B:(j + 1) * B], ones_col,
                                 a_sb[:, B:2 * B], start=False, stop=True)

                # av[d,b] = X^T a: stationary natural-X tile, 1-col moving att
                av = psV.tile([P, B], F32, tag="av")
                for b in range(B):
                    nc.tensor.matmul(av[:, b:b + 1],
                                     xn[j][:, b * W:b * W + P],
                                     a_sb[:, b:b + 1], start=True, stop=False)
                    nc.tensor.matmul(av[:, b:b + 1],
                                     xn[j][:, b * W + P:b * W + 2 * P],
                                     a_sb[:, B + b:B + b + 1],
                                     start=False, stop=True)
                avn.append(av)

            # 1/Z broadcast to all partitions via ones-row matmul
            rec = spool.tile([1, HPC * B], BF16, tag="rec")
            nc.vector.reciprocal(rec[:, :], z_ps[:, :])
            rb_ps = psZ.tile([P, HPC * B], F32, tag="zrb")
            nc.tensor.matmul(rb_ps[:, :], ones_row, rec[:, :], start=True, stop=True)
            rb_sb = spool.tile([P, HPC * B], F32, tag="rb_sb")
            nc.scalar.copy(rb_sb[:, :], rb_ps[:, :])

            fin = spool.tile([P, HPC * B], F32, tag="fin")
            o_ps = psO.tile([P, HPC * B], F32, tag="ow")      # Wv^T avn, both heads
            for j in range(HPC):
                avs = spool.tile([P, B], BF16, tag=f"avs{j}")
                nc.vector.tensor_mul(avs[:, :], avn[j][:, :],
                                     rb_sb[:, j * B:(j + 1) * B])
                nc.tensor.matmul(o_ps[:, j * B:(j + 1) * B], wv[j], avs[:, :],
                                 start=True, stop=True)
                nc.vector.tensor_add(fin[:, j * B:(j + 1) * B],
                                     o_ps[:, j * B:(j + 1) * B],
                                     cf[:, j * B:(j + 1) * B])
            nc.sync.dma_start(out=out_t[:, :], in_=fin[:, :])
    nc.finalize()
    return nc


def _get_nc():
    if "nc" not in _NC_CACHE:
        _NC_CACHE["nc"] = _build_nc()
    return _NC_CACHE["nc"]


def _pos_bias_f32():
    """t5_position_bucket exactly as the reference computes it, sliced to the
    window."""
    if "pos" not in _NC_CACHE:
        import jax.numpy as jnp
        NUM_BUCKETS, MAX_DISTANCE = 32, 128
        n = (S - 1) - jnp.arange(S)
        max_exact = NUM_BUCKETS // 2
        is_small = n < max_exact
        large = max_exact + (
            jnp.log(jnp.maximum(n, 1).astype(jnp.float32) / max_exact)
            / np.log(MAX_DISTANCE / max_exact)
            * (NUM_BUCKETS - max_exact)
        ).astype(jnp.int32)
        large = jnp.minimum(large, NUM_BUCKETS - 1)
        pos = jnp.where(is_small, n, large).astype(jnp.float32)
        _NC_CACHE["pos"] = np.asarray(pos)[CUTOFF:]  # [W]
    return _NC_CACHE["pos"]


def kernel(**inputs) -> np.ndarray:
    t = int(np.asarray(inputs["t"]))
    assert t == T, f"kernel hardcoded for t={T}, got {t}"
    content_t = np.asarray(inputs["content_t"], dtype=np.float32)
    cache = np.asarray(inputs["cache"], dtype=np.float32)
    Wq = np.asarray(inputs["Wq"], dtype=np.float32)
    bq = np.asarray(inputs["bq"], dtype=np.float32)
    Wk = np.asarray(inputs["Wk"], dtype=np.float32)
    Wv = np.asarray(inputs["Wv"], dtype=np.float32)
    bv = np.asarray(inputs["bv"], dtype=np.float32)
    pos_param = np.float32(np.asarray(inputs["pos_param"]))
    # time_mask: uniform -1e6 shift in-window (softmax-invariant); bk: adds an
    # s-constant q.bk to every in-window score (softmax-invariant). Both dropped.

    pos = _pos_bias_f32()                                   # [W]
    posb = (-pos_param * pos).astype(np.float32)            # [W]

    win = cache[:, CUTOFF:T, :].reshape(B, W - 1, H, P)     # [B, 255, H, 128]
    cnt_h = content_t.reshape(B, H, P)                      # [B, H, 128]
    wkt_full = (Wk.transpose(0, 2, 1) / np.float32(np.sqrt(128.0)))

    in_maps = []
    for c in range(NCORES):
        h0 = HPC * c
        xt_host = np.empty((HPC, P, B * W), NPBF)
        xn_host = np.empty((HPC, P, B * W), NPBF)
        for j in range(HPC):
            xwin = np.concatenate(
                [win[:, :, h0 + j, :], cnt_h[:, None, h0 + j, :]], axis=1
            )                                               # [B, 256, 128] f32
            xwb = xwin.astype(NPBF)
            xt_host[j] = xwb.transpose(2, 0, 1).reshape(P, B * W)
            xn_host[j] = (
                xwb.reshape(B, 2, P, P).transpose(2, 0, 1, 3).reshape(P, B * W)
            )
        cbw_host = np.zeros((P, CBW), NPBF)
        cbw_host[:, CB0:CB0 + HPC * B] = (
            cnt_h[:, h0:h0 + HPC, :].transpose(2, 1, 0).reshape(P, HPC * B)
        )
        for j in range(HPC):
            base = CW0 + j * 3 * P
            cbw_host[:, base:base + P] = Wq[h0 + j].astype(NPBF)
            cbw_host[:, base + P:base + 2 * P] = wkt_full[h0 + j].astype(NPBF)
            cbw_host[:, base + 2 * P:base + 3 * P] = Wv[h0 + j].astype(NPBF)
        cbw_host[:, CO0:CO0 + P] = NPBF(1.0)
        cf_host = np.zeros((P, CFW), np.float32)
        cf_host[:, 0:HPC * B] = (
            (cnt_h[:, h0:h0 + HPC, :] + bv[None, h0:h0 + HPC, :])
            .transpose(2, 1, 0).reshape(P, HPC * B)
        )
        cf_host[:, CF_BIAS] = posb[0:P]
        cf_host[:, CF_BIAS + 1] = posb[P:2 * P]
        for j in range(HPC):
            cf_host[:, CF_BQ + j] = bq[h0 + j]
        in_maps.append({
            "xt": xt_host, "xn": xn_host,
            "cbw": cbw_host, "cf": cf_host,
        })

    nc = _get_nc()
    res = run_bass_kernel_spmd(nc, in_maps, list(range(NCORES)), **_RUN_KWARGS)
    _NC_CACHE["last_results"] = res
    outs = np.stack([np.asarray(res.results[c]["out"]) for c in range(NCORES)])
    # outs: [core, d, j*8+b] -> out_full[b, (2c+j)*128 + d]
    out_full = (
        outs.reshape(NCORES, P, HPC, B).transpose(3, 0, 2, 1).reshape(B, H * P)
    )
    return np.ascontiguousarray(out_full, dtype=np.float32)


_RUN_KWARGS = {}  # test harness may set {"trace": True, "tmpdir": ...}


# revision 7
# speedup vs baseline: 3.3002x; 1.0625x over previous
"""Bass/Trainium2 kernel for nn_BiChannelAttention (single-query local-window attention).

Math (per batch b, head h, S=2049, window W=256, cutoff=S-W=1793):
  Every in-window position carries the same -1e6 time-mask shift (the reference's
  masked_fill collapses to a uniform constant), which softmax cancels; positions
  before the cutoff are -1e6 relative => weight exactly 0 in fp32. Only the last
  W positions (255 cache rows + the content row) matter.

  Per (b,h) with window rows X [W,128]:
    kq  = 64/sqrt(128) * (Wk Wq^T cnt + Wk bq)   (Wq,Wk host-folded into one
          matrix; x64 keeps fp8 kq out of subnormals; q.bk is an s-constant
          and cancels in softmax)
    sc  = X kq                                    (PE matvec, s on partitions)
    a   = exp(sc/64 - pos_param*bucket(s))        (ACT: scale+bias fused)
    av  = X^T a ; z = ones^T a                    (PE)
    out = (Wv^T av) * (1/z) + bv + cnt            (bv folded into residual)

Precision: window data X ships as fp8(e4m3) in BOTH layouts (natural [s,d] for
the AV stationary, transposed [d,s] for the scores stationary) -- ~1.05 MB/core,
DMA-bound kernel. Weights/cnt bf16, accumulation fp32 in PSUM. Host-simulated
rel err vs the fp32 reference: 8.6e-4 (gate 2e-2) -- the residual add dilutes
attention-path error ~20x.

Schedule: stationaries are per-(pair,half) 128-col tiles (FWL fp8 load), moving
operands 1-8 columns; LDW+MM pairs pipeline at ~26ns. A dummy-matmul warmup
keeps the PE HAM clock warm through the DMA window. DMA instruction count is
minimal (6 in, 2 out, ~700ns issue each) split across the two HWDGE queues.
Per-head softmax tails overlap the other head's compute.

Sharding: tensor-parallel over heads, 2 heads per core x 8 cores.
"""

import sys
import numpy as np

for _p in ("/opt/trn_rl_repo", "/root/.axon_site/_ro/trn_rl_repo"):
    if _p not in sys.path:
        sys.path.insert(0, _p)

import ml_dtypes
import concourse.bass as bass
import concourse.bacc as bacc
import concourse.mybir as mybir
from concourse.tile import TileContext
from concourse.bass_utils import run_bass_kernel_spmd

F32 = mybir.dt.float32
BF16 = mybir.dt.bfloat16
FP8 = mybir.dt.float8e4
NPBF = ml_dtypes.bfloat16
NPF8 = ml_dtypes.float8_e4m3fn
P = 128          # partitions / head_dim
B = 8            # batch
H = 16           # heads total
HPC = 2          # heads per core
NCORES = 8
T = 2048
S = T + 1
W = 256          # local attention window
CUTOFF = S - W   # 1793
NWARM = 14       # PE warmup matmuls (HAM ramp ~3.4us)
KQS = 64.0       # kq prescale (undone by exp's scale=1/KQS)

_NC_CACHE = {}

# cbw (bf16) column map: [cnt 16][per head: M | wv]
CB0 = 0
CW0 = HPC * B
CBW = CW0 + HPC * 2 * P
# cf (f32) column map: [cnt+bv 16][bias0][bias1][vb0][vb1]
CF_BIAS = HPC * B
CF_VB = CF_BIAS + 2
CFW = CF_VB + HPC


def _build_nc():
    nc = bacc.Bacc(None, target_bir_lowering=False, debug=False)
    xt_in = nc.declare_dram_parameter("xt", [HPC, P, B * W], FP8, isOutput=False)
    xn_in = nc.declare_dram_parameter("xn", [HPC, P, B * W], FP8, isOutput=False)
    cbw_in = nc.declare_dram_parameter("cbw", [P, CBW], BF16, isOutput=False)
    cf_in = nc.declare_dram_parameter("cf", [P, CFW], F32, isOutput=False)
    out_t = nc.declare_dram_parameter("out", [P, HPC * B], F32, isOutput=True)

    with TileContext(nc) as tc:
        with (
            nc.allow_low_precision(reason="fp8/bf16 pipeline validated vs reference"),
            tc.tile_pool(name="big", bufs=1) as bigp,
            tc.tile_pool(name="small", bufs=8) as spool,
            tc.tile_pool(name="ps_qk", bufs=2, space="PSUM") as psQK,
            tc.tile_pool(name="ps_z", bufs=1, space="PSUM") as psZ,
            tc.tile_pool(name="ps_s", bufs=2, space="PSUM") as psS,
            tc.tile_pool(name="ps_v", bufs=2, space="PSUM") as psV,
            tc.tile_pool(name="ps_o", bufs=1, space="PSUM") as psO,
        ):
            # ---- DMAs: sync queue cbw/xt0/xt1, scalar queue cf/xn0/xn1
            cbw = bigp.tile([P, CBW], BF16, tag="cbw")
            nc.sync.dma_start(out=cbw[:, :], in_=cbw_in[:, :])
            cf = bigp.tile([P, CFW], F32, tag="cf")
            nc.scalar.dma_start(out=cf[:, :], in_=cf_in[:, :])
            xt = []
            xn = []
            for j in range(HPC):
                t_ = bigp.tile([P, B * W], FP8, tag=f"xt{j}")
                nc.sync.dma_start(out=t_[:, :], in_=xt_in[j, :, :])
                xt.append(t_)
                n_ = bigp.tile([P, B * W], FP8, tag=f"xn{j}")
                nc.scalar.dma_start(out=n_[:, :], in_=xn_in[j, :, :])
                xn.append(n_)

            cnt_bf = cbw[:, CB0:CB0 + HPC * B]
            Mw = [cbw[:, CW0 + j * 2 * P:CW0 + j * 2 * P + P] for j in range(HPC)]
            wv = [cbw[:, CW0 + j * 2 * P + P:CW0 + j * 2 * P + 2 * P] for j in range(HPC)]

            # ones vectors via memset (no DMA): fp8 col for z, bf16 row for bcast
            ones8 = spool.tile([P, 1], FP8, tag="ones8")
            nc.vector.memset(ones8[:, :], 1.0)
            onesb = spool.tile([1, P], BF16, tag="onesb")
            nc.gpsimd.memset(onesb[:, :], 1.0)

            # ---- PE warmup while DMAs land
            wdat = spool.tile([P, P], BF16, tag="warm")
            nc.vector.memset(wdat[:, :], 0.0)
            wps = psO.tile([P, P], F32, tag="ow")
            for i in range(NWARM):
                nc.tensor.matmul(wps[:, :], wdat[:, :], wdat[:, :],
                                 start=True, stop=True)

            z_ps = psZ.tile([1, HPC * B], F32, tag="zrb")     # denominators
            fin = spool.tile([P, HPC * B], F32, tag="fin")
            for j in range(HPC):
                # kq = M^T cnt + vb  (M = 64/sqrt(128) * Wq Wk^T, host-folded)
                kq_ps = psQK.tile([P, B], F32, tag="qk")
                nc.tensor.matmul(kq_ps[:, :], Mw[j], cnt_bf[:, j * B:(j + 1) * B],
                                 start=True, stop=True)
                kq_sb = spool.tile([P, B], FP8, tag=f"kqsb{j}")
                nc.vector.tensor_scalar_add(kq_sb[:, :], kq_ps[:, :],
                                            cf[:, CF_VB + j:CF_VB + j + 1])

                # scores: per (b, s-half) stationary Xt tile, 1-col moving kq
                sc = psS.tile([P, 2 * B], F32, tag="sc")
                for b in range(B):
                    nc.tensor.matmul(sc[:, b:b + 1],
                                     xt[j][:, b * W:b * W + P],
                                     kq_sb[:, b:b + 1], start=True, stop=True)
                    nc.tensor.matmul(sc[:, B + b:B + b + 1],
                                     xt[j][:, b * W + P:b * W + 2 * P],
                                     kq_sb[:, b:b + 1], start=True, stop=True)

                # att = exp(sc/64 + bias(s)): scale+per-partition-bias fused
                a_sb = spool.tile([P, 2 * B], FP8, tag=f"att{j}")
                nc.scalar.activation(a_sb[:, 0:B], sc[:, 0:B],
                                     mybir.ActivationFunctionType.Exp,
                                     bias=cf[:, CF_BIAS:CF_BIAS + 1],
                                     scale=1.0 / KQS)
                nc.scalar.activation(a_sb[:, B:2 * B], sc[:, B:2 * B],
                                     mybir.ActivationFunctionType.Exp,
                                     bias=cf[:, CF_BIAS + 1:CF_BIAS + 2],
                                     scale=1.0 / KQS)

                # av[d,b] = X^T a (unnormalized); z = ones^T a
                av = psV.tile([P, B], F32, tag="av")
                for b in range(B):
                    nc.tensor.matmul(av[:, b:b + 1],
                                     xn[j][:, b * W:b * W + P],
                                     a_sb[:, b:b + 1], start=True, stop=False)
                    nc.tensor.matmul(av[:, b:b + 1],
                                     xn[j][:, b * W + P:b * W + 2 * P],
                                     a_sb[:, B + b:B + b + 1],
                                     start=False, stop=True)
                nc.tensor.matmul(z_ps[:, j * B:(j + 1) * B], ones8,
                                 a_sb[:, 0:B], start=True, stop=False)
                nc.tensor.matmul(z_ps[:, j * B:(j + 1) * B], ones8,
                                 a_sb[:, B:2 * B], start=False, stop=True)

                # o = Wv^T av (runs without waiting on z)
                av_sb = spool.tile([P, B], BF16, tag=f"avsb{j}")
                nc.vector.tensor_copy(av_sb[:, :], av[:, :])
                o_ps = psO.tile([P, B], F32, tag="ow")
                nc.tensor.matmul(o_ps[:, :], wv[j], av_sb[:, :],
                                 start=True, stop=True)

                # per-head softmax tail: rec -> bcast -> fin = o*rb + (cnt+bv)
                rec = spool.tile([1, B], BF16, tag=f"rec{j}")
                nc.vector.reciprocal(rec[:, :], z_ps[:, j * B:(j + 1) * B])
                rb_ps = psQK.tile([P, B], F32, tag="qk")
                nc.tensor.matmul(rb_ps[:, :], onesb, rec[:, :],
                                 start=True, stop=True)
                rb_sb = spool.tile([P, B], F32, tag=f"rbsb{j}")
                nc.scalar.copy(rb_sb[:, :], rb_ps[:, :])
                tmp = spool.tile([P, B], F32, tag=f"tmp{j}")
                nc.vector.tensor_mul(tmp[:, :], o_ps[:, :], rb_sb[:, :])
                nc.vector.tensor_add(fin[:, j * B:(j + 1) * B], tmp[:, :],
                                     cf[:, j * B:(j + 1) * B])
                if j == 0:
                    nc.scalar.dma_start(out=out_t[:, 0:B], in_=fin[:, 0:B])
                else:
                    nc.sync.dma_start(out=out_t[:, B:2 * B], in_=fin[:, B:2 * B])
    nc.finalize()
    return nc


def _get_nc():
    if "nc" not in _NC_CACHE:
        _NC_CACHE["nc"] = _build_nc()
    return _NC_CACHE["nc"]


def _pos_bias_f32():
    """t5_position_bucket exactly as the reference computes it, sliced to the
    window."""
    if "pos" not in _NC_CACHE:
        import jax.numpy as jnp
        NUM_BUCKETS, MAX_DISTANCE = 32, 128
        n = (S - 1) - jnp.arange(S)
        max_exact = NUM_BUCKETS // 2
        is_small = n < max_exact
        large = max_exact + (
            jnp.log(jnp.maximum(n, 1).astype(jnp.float32) / max_exact)
            / np.log(MAX_DISTANCE / max_exact)
            * (NUM_BUCKETS - max_exact)
        ).astype(jnp.int32)
        large = jnp.minimum(large, NUM_BUCKETS - 1)
        pos = jnp.where(is_small, n, large).astype(jnp.float32)
        _NC_CACHE["pos"] = np.asarray(pos)[CUTOFF:]  # [W]
    return _NC_CACHE["pos"]


def kernel(**inputs) -> np.ndarray:
    t = int(np.asarray(inputs["t"]))
    assert t == T, f"kernel hardcoded for t={T}, got {t}"
    content_t = np.asarray(inputs["content_t"], dtype=np.float32)
    cache = np.asarray(inputs["cache"], dtype=np.float32)
    Wq = np.asarray(inputs["Wq"], dtype=np.float32)
    bq = np.asarray(inputs["bq"], dtype=np.float32)
    Wk = np.asarray(inputs["Wk"], dtype=np.float32)
    Wv = np.asarray(inputs["Wv"], dtype=np.float32)
    bv = np.asarray(inputs["bv"], dtype=np.float32)
    pos_param = np.float32(np.asarray(inputs["pos_param"]))
    # time_mask: uniform -1e6 shift in-window (softmax-invariant); bk: adds an
    # s-constant q.bk to every in-window score (softmax-invariant). Both dropped.

    pos = _pos_bias_f32()                                   # [W]
    posb = (-pos_param * pos).astype(np.float32)            # [W]
    c = np.float32(KQS / np.sqrt(128.0))

    win = cache[:, CUTOFF:T, :].reshape(B, W - 1, H, P)     # [B, 255, H, 128]
    cnt_h = content_t.reshape(B, H, P)                      # [B, H, 128]

    in_maps = []
    for co in range(NCORES):
        h0 = HPC * co
        xt_host = np.empty((HPC, P, B * W), NPF8)
        xn_host = np.empty((HPC, P, B * W), NPF8)
        for j in range(HPC):
            xwin = np.concatenate(
                [win[:, :, h0 + j, :], cnt_h[:, None, h0 + j, :]], axis=1
            )                                               # [B, 256, 128] f32
            xwb = xwin.astype(NPF8)
            xt_host[j] = xwb.transpose(2, 0, 1).reshape(P, B * W)
            xn_host[j] = (
                xwb.reshape(B, 2, P, P).transpose(2, 0, 1, 3).reshape(P, B * W)
            )
        cbw_host = np.zeros((P, CBW), NPBF)
        cbw_host[:, CB0:CB0 + HPC * B] = (
            cnt_h[:, h0:h0 + HPC, :].transpose(2, 1, 0).reshape(P, HPC * B)
        )
        for j in range(HPC):
            base = CW0 + j * 2 * P
            h = h0 + j
            cbw_host[:, base:base + P] = (c * (Wq[h] @ Wk[h].T)).astype(NPBF)
            cbw_host[:, base + P:base + 2 * P] = Wv[h].astype(NPBF)
        cf_host = np.zeros((P, CFW), np.float32)
        cf_host[:, 0:HPC * B] = (
            (cnt_h[:, h0:h0 + HPC, :] + bv[None, h0:h0 + HPC, :])
            .transpose(2, 1, 0).reshape(P, HPC * B)
        )
        cf_host[:, CF_BIAS] = posb[0:P]
        cf_host[:, CF_BIAS + 1] = posb[P:2 * P]
        for j in range(HPC):
            cf_host[:, CF_VB + j] = c * (Wk[h0 + j] @ bq[h0 + j])
        in_maps.append({
            "xt": xt_host, "xn": xn_host,
            "cbw": cbw_host, "cf": cf_host,
        })

    nc = _get_nc()
    res = run_bass_kernel_spmd(nc, in_maps, list(range(NCORES)), **_RUN_KWARGS)
    _NC_CACHE["last_results"] = res
    outs = np.stack([np.asarray(res.results[co]["out"]) for co in range(NCORES)])
    # outs: [core, d, j*8+b] -> out_full[b, (2c+j)*128 + d]
    out_full = (
        outs.reshape(NCORES, P, HPC, B).transpose(3, 0, 2, 1).reshape(B, H * P)
    )
    return np.ascontiguousarray(out_full, dtype=np.float32)


_RUN_KWARGS = {}  # test harness may set {"trace": True, "tmpdir": ...}


# revision 9
# speedup vs baseline: 3.3256x; 1.0077x over previous
"""Bass/Trainium2 kernel for nn_BiChannelAttention (single-query local-window attention).

Math (per batch b, head h, S=2049, window W=256, cutoff=S-W=1793):
  Every in-window position carries the same -1e6 time-mask shift (the reference's
  masked_fill collapses to a uniform constant), which softmax cancels; positions
  before the cutoff are -1e6 relative => weight exactly 0 in fp32. Only the last
  W positions (255 cache rows + the content row) matter.

  Per (b,h) with window rows X [W,128]:
    kq  = 64/sqrt(128) * (Wk Wq^T cnt + Wk bq)   (Wq,Wk host-folded; x64 keeps
          fp8 kq out of subnormals; q.bk is an s-constant, cancels in softmax)
    sc  = X kq                                    (PE matvec, s on partitions)
    a   = exp(sc/64 - pos_param*bucket(s))        (ACT: scale+bias fused)
    av  = X^T a ; z^T = a^T ones                  (PE; z lands b-on-partitions)
    outT = (av^T Wv) * (1/z)[b] + (bv + cnt)^T    (transposed output: per-b 1/z
          becomes a per-partition tensor_scalar -- no broadcast matmul needed)

Precision: window data X ships as fp8(e4m3) in BOTH layouts (natural [s,d] for
the AV stationary, transposed [d,s] for the scores stationary); weights/cnt
bf16, accumulation fp32 in PSUM. Host-simulated rel err vs the fp32 reference:
~9e-4 (gate 2e-2) -- the residual add dilutes attention-path error ~20x.

Perf structure (from NTFF traces): each dma_start costs ~650ns issue plus
serialized per-descriptor dispatch on its HWDGE queue, so inputs ship as ONE
merged byte-buffer DMA per queue (sync + scalar), bitcast into typed regions
on SBUF; the output is built transposed [16,128] so the store is 16
descriptors instead of 128. Stationaries are per-(pair,half) 128-col fp8
tiles (FWL); LDW+MM pairs pipeline at ~26ns. Dummy-matmul warmup keeps the
PE HAM clock warm through the DMA window. DVE ops are ordered so the strict
FIFO never blocks a later head's prerequisites behind an earlier head's tail.

Sharding: tensor-parallel over heads, 2 heads per core x 8 cores.
"""

import sys
import numpy as np

for _p in ("/opt/trn_rl_repo", "/root/.axon_site/_ro/trn_rl_repo"):
    if _p not in sys.path:
        sys.path.insert(0, _p)

import ml_dtypes
import concourse.bass as bass
import concourse.bacc as bacc
import concourse.mybir as mybir
from concourse.tile import TileContext
from concourse.bass_utils import run_bass_kernel_spmd

F32 = mybir.dt.float32
BF16 = mybir.dt.bfloat16
FP8 = mybir.dt.float8e4
U8 = mybir.dt.uint8
NPBF = ml_dtypes.bfloat16
NPF8 = ml_dtypes.float8_e4m3fn
P = 128          # partitions / head_dim
B = 8            # batch
H = 16           # heads total
HPC = 2          # heads per core
NCORES = 8
T = 2048
S = T + 1
W = 256          # local attention window
CUTOFF = S - W   # 1793
NWARM = 26       # PE warmup matmuls (HAM ramp ~3.4us, covers DMA window)
KQS = 64.0       # kq prescale (undone by exp's scale=1/KQS)

_NC_CACHE = {}

# Buffer A (sync queue), bytes per partition row:
#   [cbw bf16: cnt 16 cols | per head (M 128 | wv 128) => 528 cols = 1056 B]
#   [xt0 fp8 2048 B][xt1 fp8 2048 B]
CBW = HPC * B + HPC * 2 * P                    # 528 bf16 cols
A_XT0 = 2 * CBW                                # byte offsets
A_XT1 = A_XT0 + B * W
A_BYTES = A_XT1 + B * W
# Buffer B (scalar queue):
#   [cf f32 4 cols = 16 B: bias0|bias1|vb0|vb1][cntT f32 2x128 cols = 1024 B
#    (per head, partitions 0-7 hold (cnt+bv)^T, rest zero)][xn0 2048][xn1 2048]
CFW = 4
B_CNTT = 4 * CFW
B_XN0 = B_CNTT + HPC * 4 * P
B_XN1 = B_XN0 + B * W
B_BYTES = B_XN1 + B * W


def _build_nc():
    nc = bacc.Bacc(None, target_bir_lowering=False, debug=False)
    ina_in = nc.declare_dram_parameter("ina", [P, A_BYTES], U8, isOutput=False)
    inb_in = nc.declare_dram_parameter("inb", [P, B_BYTES], U8, isOutput=False)
    out_t = nc.declare_dram_parameter("out", [HPC, B, P], F32, isOutput=True)

    with TileContext(nc) as tc:
        with (
            nc.allow_low_precision(reason="fp8/bf16 pipeline validated vs reference"),
            tc.tile_pool(name="big", bufs=1) as bigp,
            tc.tile_pool(name="small", bufs=8) as spool,
            tc.tile_pool(name="ps_qk", bufs=2, space="PSUM") as psQK,
            tc.tile_pool(name="ps_s", bufs=2, space="PSUM") as psS,
            tc.tile_pool(name="ps_v", bufs=2, space="PSUM") as psV,
            tc.tile_pool(name="ps_zo", bufs=2, space="PSUM") as psZO,
        ):
            # ---- one merged DMA per HWDGE queue
            ina = bigp.tile([P, A_BYTES], U8, tag="ina")
            nc.sync.dma_start(out=ina[:, :], in_=ina_in[:, :])
            inb = bigp.tile([P, B_BYTES], U8, tag="inb")
            nc.scalar.dma_start(out=inb[:, :], in_=inb_in[:, :])

            cbw = ina[:, 0:2 * CBW].bitcast(BF16)
            cnt_bf = cbw[:, 0:HPC * B]
            Mw = [cbw[:, HPC * B + j * 2 * P:HPC * B + j * 2 * P + P]
                  for j in range(HPC)]
            wv = [cbw[:, HPC * B + j * 2 * P + P:HPC * B + j * 2 * P + 2 * P]
                  for j in range(HPC)]
            xt = [ina[:, A_XT0:A_XT0 + B * W].bitcast(FP8),
                  ina[:, A_XT1:A_XT1 + B * W].bitcast(FP8)]
            cf = inb[:, 0:4 * CFW].bitcast(F32)
            cntT = [inb[:, B_CNTT + j * 4 * P:B_CNTT + (j + 1) * 4 * P]
                    .bitcast(F32) for j in range(HPC)]  # rows 0-7 used
            xn = [inb[:, B_XN0:B_XN0 + B * W].bitcast(FP8),
                  inb[:, B_XN1:B_XN1 + B * W].bitcast(FP8)]

            ones8 = spool.tile([P, 1], FP8, tag="ones8")
            nc.vector.memset(ones8[:, :], 1.0)

            # ---- PE warmup while the DMAs land
            wdat = spool.tile([P, P], BF16, tag="warm")
            nc.vector.memset(wdat[:, :], 0.0)
            wps = psZO.tile([P, P], F32, tag="zo")
            for i in range(NWARM):
                nc.tensor.matmul(wps[:, :], wdat[:, :], wdat[:, :],
                                 start=True, stop=True)

            # ---- phase 1: kq for both heads (keeps DVE FIFO unblocked)
            kq_sb = []
            for j in range(HPC):
                kq_ps = psQK.tile([P, B], F32, tag="qk")
                nc.tensor.matmul(kq_ps[:, :], Mw[j], cnt_bf[:, j * B:(j + 1) * B],
                                 start=True, stop=True)
                k_sb = spool.tile([P, B], FP8, tag=f"kqsb{j}")
                nc.vector.tensor_scalar_add(k_sb[:, :], kq_ps[:, :],
                                            cf[:, 2 + j:3 + j])
                kq_sb.append(k_sb)

            # ---- phase 2: scores + exp per head
            att = []
            for j in range(HPC):
                sc = psS.tile([P, 2 * B], F32, tag="sc")
                for b in range(B):
                    nc.tensor.matmul(sc[:, b:b + 1],
                                     xt[j][:, b * W:b * W + P],
                                     kq_sb[j][:, b:b + 1], start=True, stop=True)
                    nc.tensor.matmul(sc[:, B + b:B + b + 1],
                                     xt[j][:, b * W + P:b * W + 2 * P],
                                     kq_sb[j][:, b:b + 1], start=True, stop=True)
                a_sb = spool.tile([P, 2 * B], FP8, tag=f"att{j}")
                nc.scalar.activation(a_sb[:, 0:B], sc[:, 0:B],
                                     mybir.ActivationFunctionType.Exp,
                                     bias=cf[:, 0:1], scale=1.0 / KQS)
                nc.scalar.activation(a_sb[:, B:2 * B], sc[:, B:2 * B],
                                     mybir.ActivationFunctionType.Exp,
                                     bias=cf[:, 1:2], scale=1.0 / KQS)
                att.append(a_sb)

            # ---- phase 3/4: av, z^T, o^T, normalized transposed output
            for j in range(HPC):
                a_sb = att[j]
                av = psV.tile([P, B], F32, tag="av")
                for b in range(B):
                    nc.tensor.matmul(av[:, b:b + 1],
                                     xn[j][:, b * W:b * W + P],
                                     a_sb[:, b:b + 1], start=True, stop=False)
                    nc.tensor.matmul(av[:, b:b + 1],
                                     xn[j][:, b * W + P:b * W + 2 * P],
                                     a_sb[:, B + b:B + b + 1],
                                     start=False, stop=True)
                # z^T [8,1]: stationary a (8 cols), moving ones
                zt_ps = psZO.tile([B, 1], F32, tag="zo")
                nc.tensor.matmul(zt_ps[:, :], a_sb[:, 0:B], ones8,
                                 start=True, stop=False)
                nc.tensor.matmul(zt_ps[:, :], a_sb[:, B:2 * B], ones8,
                                 start=False, stop=True)

                av_sb = spool.tile([P, B], BF16, tag=f"avsb{j}")
                nc.vector.tensor_copy(av_sb[:, :], av[:, :])
                rec_t = spool.tile([B, 1], F32, tag=f"rec{j}")
                nc.vector.reciprocal(rec_t[:, :], zt_ps[:, :])

                # o^T [8,128] = av^T Wv  (stationary av_sb, moving wv)
                ot_ps = psZO.tile([B, P], F32, tag="zo")
                nc.tensor.matmul(ot_ps[:, :], av_sb[:, :], wv[j],
                                 start=True, stop=True)
                tmp_t = spool.tile([B, P], F32, tag=f"tmp{j}")
                nc.vector.tensor_scalar_mul(tmp_t[:, :], ot_ps[:, :], rec_t[:, :])
                fin_t = spool.tile([B, P], F32, tag=f"fin{j}")
                nc.vector.tensor_add(fin_t[:, :], tmp_t[:, :],
                                     cntT[j][0:B, :])
                if j == 0:
                    nc.scalar.dma_start(out=out_t[0], in_=fin_t[:, :])
                else:
                    nc.sync.dma_start(out=out_t[1], in_=fin_t[:, :])
    nc.finalize()
    return nc


def _get_nc():
    if "nc" not in _NC_CACHE:
        _NC_CACHE["nc"] = _build_nc()
    return _NC_CACHE["nc"]


def _pos_bias_f32():
    """t5_position_bucket exactly as the reference computes it, sliced to the
    window."""
    if "pos" not in _NC_CACHE:
        import jax.numpy as jnp
        NUM_BUCKETS, MAX_DISTANCE = 32, 128
        n = (S - 1) - jnp.arange(S)
        max_exact = NUM_BUCKETS // 2
        is_small = n < max_exact
        large = max_exact + (
            jnp.log(jnp.maximum(n, 1).astype(jnp.float32) / max_exact)
            / np.log(MAX_DISTANCE / max_exact)
            * (NUM_BUCKETS - max_exact)
        ).astype(jnp.int32)
        large = jnp.minimum(large, NUM_BUCKETS - 1)
        pos = jnp.where(is_small, n, large).astype(jnp.float32)
        _NC_CACHE["pos"] = np.asarray(pos)[CUTOFF:]  # [W]
    return _NC_CACHE["pos"]


def kernel(**inputs) -> np.ndarray:
    t = int(np.asarray(inputs["t"]))
    assert t == T, f"kernel hardcoded for t={T}, got {t}"
    content_t = np.asarray(inputs["content_t"], dtype=np.float32)
    cache = np.asarray(inputs["cache"], dtype=np.float32)
    Wq = np.asarray(inputs["Wq"], dtype=np.float32)
    bq = np.asarray(inputs["bq"], dtype=np.float32)
    Wk = np.asarray(inputs["Wk"], dtype=np.float32)
    Wv = np.asarray(inputs["Wv"], dtype=np.float32)
    bv = np.asarray(inputs["bv"], dtype=np.float32)
    pos_param = np.float32(np.asarray(inputs["pos_param"]))
    # time_mask: uniform -1e6 shift in-window (softmax-invariant); bk: adds an
    # s-constant q.bk to every in-window score (softmax-invariant). Both dropped.

    pos = _pos_bias_f32()                                   # [W]
    posb = (-pos_param * pos).astype(np.float32)            # [W]
    c = np.float32(KQS / np.sqrt(128.0))

    win = cache[:, CUTOFF:T, :].reshape(B, W - 1, H, P)     # [B, 255, H, 128]
    cnt_h = content_t.reshape(B, H, P)                      # [B, H, 128]

    in_maps = []
    for co in range(NCORES):
        h0 = HPC * co
        ina = np.zeros((P, A_BYTES), np.uint8)
        inb = np.zeros((P, B_BYTES), np.uint8)
        cbw = np.zeros((P, CBW), NPBF)
        cbw[:, 0:HPC * B] = (
            cnt_h[:, h0:h0 + HPC, :].transpose(2, 1, 0).reshape(P, HPC * B)
        )
        for j in range(HPC):
            base = HPC * B + j * 2 * P
            h = h0 + j
            cbw[:, base:base + P] = (c * (Wq[h] @ Wk[h].T)).astype(NPBF)
            cbw[:, base + P:base + 2 * P] = Wv[h].astype(NPBF)
        ina[:, 0:2 * CBW] = cbw.view(np.uint8)
        for j in range(HPC):
            xwin = np.concatenate(
                [win[:, :, h0 + j, :], cnt_h[:, None, h0 + j, :]], axis=1
            )                                               # [B, 256, 128] f32
            xwb = xwin.astype(NPF8)
            off = A_XT0 if j == 0 else A_XT1
            ina[:, off:off + B * W] = (
                xwb.transpose(2, 0, 1).reshape(P, B * W).view(np.uint8)
            )
            offn = B_XN0 if j == 0 else B_XN1
            inb[:, offn:offn + B * W] = (
                xwb.reshape(B, 2, P, P).transpose(2, 0, 1, 3)
                .reshape(P, B * W).view(np.uint8)
            )
        cfh = np.zeros((P, CFW), np.float32)
        cfh[:, 0] = posb[0:P]
        cfh[:, 1] = posb[P:2 * P]
        for j in range(HPC):
            cfh[:, 2 + j] = c * (Wk[h0 + j] @ bq[h0 + j])
        inb[:, 0:4 * CFW] = cfh.view(np.uint8)
        for j in range(HPC):
            cntT = np.zeros((P, P), np.float32)
            cntT[0:B, :] = cnt_h[:, h0 + j, :] + bv[None, h0 + j, :]
            inb[:, B_CNTT + j * 4 * P:B_CNTT + (j + 1) * 4 * P] = (
                cntT.view(np.uint8)
            )
        in_maps.append({"ina": ina, "inb": inb})

    nc = _get_nc()
    res = run_bass_kernel_spmd(nc, in_maps, list(range(NCORES)), **_RUN_KWARGS)
    _NC_CACHE["last_results"] = res
    outs = np.stack([np.asarray(res.results[co]["out"]) for co in range(NCORES)])
    # outs: [core, j, b, d] -> out_full[b, (2c+j)*128 + d]
    out_full = outs.transpose(2, 0, 1, 3).reshape(B, H * P)
    return np.ascontiguousarray(out_full, dtype=np.float32)


_RUN_KWARGS = {}  # test harness may set {"trace": True, "tmpdir": ...}
